# revision 1
# baseline (speedup 1.0000x reference)
"""Trainium2 Bass kernel for nn_DIYloss_1709396984424.

Loss: for binary labels, mean over (one, zero) pairs of (1 + p[l] - p[k])^2
where p = sigmoid(pred_Y). The L^2 pairwise sum has an exact closed form in
O(L) masked reductions:

    pair_sum = n1*Q2 - 2*s1*Q1 + n0*s2
      s1 = sum(m1*p), s2 = sum(m1*p^2)
      s0_1 = sum(p) - s1, s0_2 = sum(p^2) - s2, n0 = L - n1
      Q1 = n0 + s0_1,  Q2 = n0 + 2*s0_1 + s0_2

    loss = pair_sum / max(n1*n0, 1) + [n1 == 0] * mean(p^2)
    (pair_sum is exactly 0 when n1 == 0, so the blend needs no subtraction)

Each of the 8 cores receives the full (replicated) input and computes the
full scalar on-device; core 0's output is returned. The two inputs are
packed host-side into one [128,128] f32 buffer (int32 labels bitcast into
the second half) so a single DMA brings everything in. Per-core work: one
32 KiB DMA, ACT sigmoid/squares with fused row-sums, a few DVE ops, one
tiny PE matmul for the partition-axis sum, and a short scalar epilogue.
"""

import numpy as np

try:
    import concourse.bass as bass  # noqa: F401
except ImportError:  # pragma: no cover - grading env should have it on path
    import sys

    sys.path.insert(0, "/opt/trn_rl_repo")
    import concourse.bass as bass  # noqa: F401

import concourse.tile as tile
from concourse import bacc, mybir
from concourse.bass_utils import run_bass_kernel_spmd

L = 8192
P = 128
F = L // P  # 64
N_CORES = 8

_f32 = mybir.dt.float32
_i32 = mybir.dt.int32
_Alu = mybir.AluOpType
_Act = mybir.ActivationFunctionType

_built = None


def _build_tile():
    nc = bacc.Bacc(
        "TRN2", debug=False, target_bir_lowering=False, num_devices=N_CORES
    )
    # cols 0:F = pred_Y (f32), cols F:2F = true_Y (int32 bitcast to f32)
    xin_d = nc.dram_tensor("xin", [P, 2 * F], _f32, kind="ExternalInput")
    out_d = nc.dram_tensor("out", [1, 1], _f32, kind="ExternalOutput")

    with tile.TileContext(nc) as tc:
        with (
            tc.tile_pool(name="sbuf", bufs=1) as pool,
            tc.tile_pool(name="psum", bufs=1, space="PSUM") as psum,
        ):
            xt = pool.tile([P, 2 * F], _f32)
            nc.sync.dma_start(xt[:], xin_d[:])
            pred_v = xt[:, 0:F]
            true_v = xt[:, F : 2 * F].bitcast(_i32)

            p = pool.tile([P, F], _f32)
            p2 = pool.tile([P, F], _f32)
            m1 = pool.tile([P, F], _f32)
            mp = pool.tile([P, F], _f32)
            mp2 = pool.tile([P, F], _f32)
            stats = pool.tile([P, 8], _f32)
            ones = pool.tile([P, 1], _f32)

            # stats columns (per-partition row sums):
            # 0: sum(p^2)  1: n1  2: s1=sum(m1*p)  3: s2=sum((m1*p)^2)
            # 4: s0_1=sum(p-m1*p)  5: s0_2=sum(p^2-(m1*p)^2)
            # s0_* are summed from element-wise differences (not total minus
            # total) so they are exactly 0 when the mask is degenerate.
            # Every DVE producer op carries its row-sum via accum_out; ACT
            # only does the sigmoid.
            # NOTE: only mybir-level ops here; raw bass_isa opcodes (e.g.
            # tensor_tensor_reduce) crash the neuronx-cc/PJRT execution path.
            m0p = pool.tile([P, F], _f32)
            m0p2 = pool.tile([P, F], _f32)
            nc.scalar.activation(p[:], pred_v, _Act.Sigmoid)
            # m1 = float(true), n1 = rowsum(m1)
            nc.vector.tensor_copy(m1[:], true_v)  # int32 -> f32 cast, values 0/1
            nc.vector.tensor_reduce(
                stats[:, 1:2], m1[:], axis=mybir.AxisListType.X, op=_Alu.add
            )
            # mp = m1*p, s1 = rowsum(mp)
            nc.vector.scalar_tensor_tensor(
                out=mp[:], in0=m1[:], scalar=1.0, in1=p[:],
                op0=_Alu.mult, op1=_Alu.mult, accum_out=stats[:, 2:3],
            )
            # p2 = p*p, t2 = rowsum(p2)
            nc.vector.scalar_tensor_tensor(
                out=p2[:], in0=p[:], scalar=1.0, in1=p[:],
                op0=_Alu.mult, op1=_Alu.mult, accum_out=stats[:, 0:1],
            )
            # mp2 = mp*mp, s2 = rowsum(mp2); m1 is 0/1 so (m1*p)^2 == m1*p^2
            nc.vector.scalar_tensor_tensor(
                out=mp2[:], in0=mp[:], scalar=1.0, in1=mp[:],
                op0=_Alu.mult, op1=_Alu.mult, accum_out=stats[:, 3:4],
            )
            nc.vector.scalar_tensor_tensor(
                out=m0p[:], in0=mp[:], scalar=-1.0, in1=p[:],
                op0=_Alu.mult, op1=_Alu.add, accum_out=stats[:, 4:5],
            )
            nc.vector.scalar_tensor_tensor(
                out=m0p2[:], in0=mp2[:], scalar=-1.0, in1=p2[:],
                op0=_Alu.mult, op1=_Alu.add, accum_out=stats[:, 5:6],
            )

            # Partition-axis reduction: ones^T [128,1] @ stats[:, 0:6] -> [1,6]
            nc.vector.memset(ones[:], 1.0)
            acc = psum.tile([1, 8], _f32)
            nc.tensor.matmul(
                acc[0:1, 0:6], ones[:], stats[:, 0:6], start=True, stop=True
            )
            # HW rule NCC_IBVF027: at most one non-scalar PSUM operand per
            # instruction — land the totals in SBUF once, then stay in SBUF.
            r = pool.tile([1, 8], _f32)
            nc.vector.tensor_copy(r[0:1, 0:6], acc[0:1, 0:6])
            t2 = r[0:1, 0:1]  # sum(p^2)
            n1 = r[0:1, 1:2]
            s1 = r[0:1, 2:3]
            s2 = r[0:1, 3:4]
            s0_1 = r[0:1, 4:5]
            s0_2 = r[0:1, 5:6]

            w = pool.tile([1, 16], _f32)

            def c(i):
                return w[0:1, i : i + 1]

            # ACT (idle by now) computes the two totals-only affine terms.
            nc.scalar.activation(
                c(2), n1, _Act.Copy, bias=float(L), scale=-1.0
            )  # n0 = L - n1
            nc.scalar.activation(
                c(12), t2, _Act.Copy, bias=0.0, scale=1.0 / L
            )  # zero_loss = mean(p^2)
            nc.vector.tensor_add(c(3), c(2), s0_1)  # Q1 = n0 + s0_1
            nc.vector.tensor_add(c(4), c(3), s0_1)  # n0 + 2*s0_1
            nc.vector.tensor_add(c(5), c(4), s0_2)  # Q2
            # G = (s1*2)*Q1
            nc.vector.scalar_tensor_tensor(
                out=c(6), in0=s1, scalar=2.0, in1=c(3), op0=_Alu.mult, op1=_Alu.mult
            )
            # H = (n0*s2) - G
            nc.vector.scalar_tensor_tensor(
                out=c(7), in0=c(2), scalar=s2, in1=c(6),
                op0=_Alu.mult, op1=_Alu.subtract,
            )
            # pair_sum = (n1*Q2) + H
            nc.vector.scalar_tensor_tensor(
                out=c(8), in0=n1, scalar=c(5), in1=c(7),
                op0=_Alu.mult, op1=_Alu.add,
            )
            # denom = max(n1*n0, 1); integers so equals where(n1*n0>0, n1*n0, 1)
            nc.vector.scalar_tensor_tensor(
                out=c(9), in0=c(2), scalar=n1, in1=ones[0:1, 0:1],
                op0=_Alu.mult, op1=_Alu.max,
            )
            nc.vector.reciprocal(c(10), c(9))
            nc.vector.tensor_mul(c(11), c(8), c(10))  # pair_loss
            # flag = [n1 == 0]
            nc.vector.tensor_scalar(
                out=c(13), in0=n1, scalar1=0.0, scalar2=None, op0=_Alu.is_equal
            )
            # out = pair_loss + flag*zero_loss (pair_loss == 0 exactly when n1==0)
            nc.vector.scalar_tensor_tensor(
                out=c(14), in0=c(13), scalar=c(12), in1=c(11),
                op0=_Alu.mult, op1=_Alu.add,
            )

            # 4-byte result: SP register load + direct store to DRAM. Skips
            # the whole HWDGE path (trigger + transfer + 900ns DMA-sem
            # propagation) that a dma_start would pay. Bitcast because
            # TENSOR_LOAD moves raw bytes through an untyped register.
            with tc.tile_critical():
                reg = nc.sync.alloc_register()
                nc.sync.reg_load(reg, c(14).bitcast(_i32))
                nc.sync.store(out_d[0:1, 0:1].bitcast(_i32), reg)

    nc.compile()
    return nc


def _build_raw2():
    nc = bacc.Bacc(
        "TRN2", debug=False, target_bir_lowering=False, num_devices=N_CORES
    )
    xin_d = nc.dram_tensor("xin", [P, 2 * F], _f32, kind="ExternalInput")
    out_d = nc.dram_tensor("out", [1, 1], _f32, kind="ExternalOutput")
    X = mybir.AxisListType.X

    with (
        nc.sbuf_tensor("xt", [P, 2 * F], _f32) as xt,
        nc.sbuf_tensor("p", [P, F], _f32) as p,
        nc.sbuf_tensor("p2", [P, F], _f32) as p2,
        nc.sbuf_tensor("m1", [P, F], _f32) as m1,
        nc.sbuf_tensor("mp", [P, F], _f32) as mp,
        nc.sbuf_tensor("mp2", [P, F], _f32) as mp2,
        nc.sbuf_tensor("m0p", [P, F], _f32) as m0p,
        nc.sbuf_tensor("m0p2", [P, F], _f32) as m0p2,
        nc.sbuf_tensor("stats", [P, 8], _f32) as stats,
        nc.sbuf_tensor("ones", [P, 1], _f32) as ones,
        nc.sbuf_tensor("rw", [1, 32], _f32) as rw,
        nc.psum_tensor("acc", [1, 8], _f32) as acc,
        nc.semaphore("s_in") as s_in,
        nc.semaphore("s_act") as s_act,
        nc.semaphore("s_dve") as s_dve,
        nc.semaphore("s_pe") as s_pe,
        nc.Block() as block,
    ):
        pred_v = xt[:, 0:F]
        true_v = xt[:, F : 2 * F].bitcast(_i32)
        t2 = rw[0:1, 0:1]
        n1 = rw[0:1, 1:2]
        s1 = rw[0:1, 2:3]
        s2 = rw[0:1, 3:4]
        s0_1 = rw[0:1, 4:5]
        s0_2 = rw[0:1, 5:6]

        def c(i):
            return rw[0:1, 8 + i : 9 + i]

        @block.sync
        def _(sp):
            sp.dma_start(xt[:], xin_d[:]).then_inc(s_in, 16)
            reg = sp.alloc_register()
            sp.reg_load(reg, c(12).bitcast(_i32))._wait_ge(s_dve, 22)
            sp.store(out_d[0:1, 0:1].bitcast(_i32), reg)

        @block.scalar
        def _(act):
            act.wait_ge(s_in, 16)
            nc.scalar.activation(p[:], pred_v, _Act.Sigmoid).then_inc(s_act, 1)

        @block.vector
        def _(dve):
            nc.vector.memset(ones[:], 1.0).then_inc(s_dve, 1)               # 1
            nc.vector.tensor_copy(m1[:], true_v)._wait_ge(s_in, 16).then_inc(s_dve, 1)  # 2
            nc.vector.tensor_reduce(
                stats[:, 1:2], m1[:], axis=X, op=_Alu.add
            )._wait_ge(s_dve, 2).then_inc(s_dve, 1)                         # 3
            nc.vector.scalar_tensor_tensor(
                out=mp[:], in0=m1[:], scalar=1.0, in1=p[:],
                op0=_Alu.mult, op1=_Alu.mult, accum_out=stats[:, 2:3],
            )._wait_ge(s_act, 1).then_inc(s_dve, 1)                         # 4
            nc.vector.scalar_tensor_tensor(
                out=p2[:], in0=p[:], scalar=1.0, in1=p[:],
                op0=_Alu.mult, op1=_Alu.mult, accum_out=stats[:, 0:1],
            ).then_inc(s_dve, 1)                                            # 5
            nc.vector.scalar_tensor_tensor(
                out=mp2[:], in0=mp[:], scalar=1.0, in1=mp[:],
                op0=_Alu.mult, op1=_Alu.mult, accum_out=stats[:, 3:4],
            )._wait_ge(s_dve, 4).then_inc(s_dve, 1)                         # 6
            nc.vector.scalar_tensor_tensor(
                out=m0p[:], in0=mp[:], scalar=-1.0, in1=p[:],
                op0=_Alu.mult, op1=_Alu.add, accum_out=stats[:, 4:5],
            ).then_inc(s_dve, 1)                                            # 7
            nc.vector.scalar_tensor_tensor(
                out=m0p2[:], in0=mp2[:], scalar=-1.0, in1=p2[:],
                op0=_Alu.mult, op1=_Alu.add, accum_out=stats[:, 5:6],
            )._wait_ge(s_dve, 6).then_inc(s_dve, 1)                         # 8

            nc.vector.tensor_copy(
                rw[0:1, 0:6], acc[0:1, 0:6]
            )._wait_ge(s_pe, 1).then_inc(s_dve, 1)                          # 9
            nc.vector.tensor_scalar(
                out=c(0), in0=n1, scalar1=-1.0, scalar2=float(L),
                op0=_Alu.mult, op1=_Alu.add,
            )._wait_ge(s_dve, 9).then_inc(s_dve, 1)                         # 10 n0
            nc.vector.tensor_scalar(
                out=c(1), in0=t2, scalar1=1.0 / L, scalar2=None, op0=_Alu.mult
            ).then_inc(s_dve, 1)                                            # 11 zl
            nc.vector.tensor_add(c(2), c(0), s0_1)._wait_ge(s_dve, 10).then_inc(s_dve, 1)  # 12 Q1
            nc.vector.tensor_add(c(3), c(2), s0_1)._wait_ge(s_dve, 12).then_inc(s_dve, 1)  # 13
            nc.vector.tensor_add(c(4), c(3), s0_2)._wait_ge(s_dve, 13).then_inc(s_dve, 1)  # 14 Q2
            nc.vector.scalar_tensor_tensor(
                out=c(5), in0=s1, scalar=2.0, in1=c(2),
                op0=_Alu.mult, op1=_Alu.mult,
            ).then_inc(s_dve, 1)                                            # 15 G
            nc.vector.scalar_tensor_tensor(
                out=c(6), in0=c(0), scalar=s2, in1=c(5),
                op0=_Alu.mult, op1=_Alu.subtract,
            )._wait_ge(s_dve, 15).then_inc(s_dve, 1)                        # 16 H
            nc.vector.scalar_tensor_tensor(
                out=c(7), in0=n1, scalar=c(4), in1=c(6),
                op0=_Alu.mult, op1=_Alu.add,
            )._wait_ge(s_dve, 16).then_inc(s_dve, 1)                        # 17 pair
            nc.vector.scalar_tensor_tensor(
                out=c(8), in0=c(0), scalar=n1, in1=ones[0:1, 0:1],
                op0=_Alu.mult, op1=_Alu.max,
            ).then_inc(s_dve, 1)                                            # 18 denom
            nc.vector.reciprocal(c(9), c(8))._wait_ge(s_dve, 18).then_inc(s_dve, 1)  # 19
            nc.vector.tensor_mul(c(10), c(7), c(9))._wait_ge(s_dve, 19).then_inc(s_dve, 1)  # 20 pl
            nc.vector.tensor_scalar(
                out=c(11), in0=n1, scalar1=0.0, scalar2=None, op0=_Alu.is_equal
            ).then_inc(s_dve, 1)                                            # 21 flag
            nc.vector.scalar_tensor_tensor(
                out=c(12), in0=c(11), scalar=c(1), in1=c(10),
                op0=_Alu.mult, op1=_Alu.add,
            )._wait_ge(s_dve, 21).then_inc(s_dve, 1)                        # 22 out

        s_in_num, s_pe_num = s_in.num, s_pe.num

        @block.tensor
        def _(pe):
            pe.wait_ge(s_dve, 8)
            nc.tensor.matmul(
                acc[0:1, 0:6], ones[:], stats[:, 0:6], start=True, stop=True
            ).then_inc(s_pe, 1)

    # self-cleaning tail: one all-engine barrier (the recognized ALL_ENGINES
    # pair), then DMA-queue + semaphore reset. reset()'s second barrier is
    # only needed mid-program; at program end the next execution cannot start
    # until every engine (including the clearing one) has finished.
    sem_range = range(s_in_num, s_pe_num + 1)
    nc.all_engine_barrier()
    nc.gpsimd.dma_reset(sem_range)
    nc.gpsimd.sem_clear(sem_range)
    nc.compile()
    return nc


# raw builder is ~2% faster and equally re-execution-safe (framework reset tail)
_build = _build_raw2


def _pack(pred_Y, true_Y):
    xin = np.empty((P, 2 * F), dtype=np.float32)
    xin[:, 0:F] = np.ascontiguousarray(pred_Y, dtype=np.float32).reshape(P, F)
    xin[:, F : 2 * F] = (
        np.ascontiguousarray(true_Y, dtype=np.int32).reshape(P, F).view(np.float32)
    )
    return xin


def _run(pred_Y, true_Y, **hw_kwargs):
    global _built
    if _built is None:
        _built = _build()
    in_map = {"xin": _pack(pred_Y, true_Y)}
    res = run_bass_kernel_spmd(
        _built, [in_map] * N_CORES, list(range(N_CORES)), **hw_kwargs
    )
    out = np.asarray(res.results[0]["out"], dtype=np.float32).reshape(())
    return out, res


def kernel(pred_Y, true_Y):
    out, _ = _run(pred_Y, true_Y)
    return out



# revision 9
# speedup vs baseline: 1.2467x; 1.2467x over previous
"""Trainium2 Bass kernel for nn_DIYloss_1709396984424.

Loss: for binary labels, mean over (one, zero) pairs of (1 + p[l] - p[k])^2
where p = sigmoid(pred_Y). With q = 1 - p, each pair term is
(q_k + p_l)^2, so the L^2 sum has the closed form

    pair_sum = n0*alpha + 2*beta*gamma + n1*delta
      n1 = sum(m),        n0 = L - n1        (m = one-mask)
      s1 = sum(m*p),      s2 = sum(m*p^2)
      gamma = sum((1-m)*p),  delta = sum((1-m)*p^2)
      alpha = sum(m*q^2) = n1 - 2*s1 + s2,  beta = sum(m*q) = n1 - s1

    loss = pair_sum / max(n1*n0, 1)

Each of the 8 cores receives the full (replicated) input and computes the
full scalar on-device; core 0's output is returned. The two inputs are
packed host-side into one [128,128] f32 buffer (int32 labels bitcast into
the second half) so a single DMA brings everything in.

Schedule (per core): the SP DMA trigger is hoisted in front of the
framework preamble barrier so the ~640ns preamble (const-AP memsets +
all-engine barrier) hides entirely under the ~2.4us DMA latency. ACT does
the sigmoid; DVE produces the five masked row-sum columns with fused
accum_out; one tiny PE matmul reduces the partition axis; the epilogue is
8 small DVE ops (the 3-term pair_sum is one fused multiply+row-accum over
a 3-lane vector) plus the denominator on the Pool engine. The 4-byte
result goes out via SP register store (no DMA). Tail: SP's store bumps a
semaphore and Pool alone does dma_reset + sem_clear — no all-engine
barrier waves.
"""

import numpy as np

try:
    import concourse.bass as bass  # noqa: F401
except ImportError:  # pragma: no cover - grading env should have it on path
    import sys

    sys.path.insert(0, "/opt/trn_rl_repo")
    import concourse.bass as bass  # noqa: F401

from concourse import bacc, mybir
from concourse.bass_utils import run_bass_kernel_spmd

L = 8192
P = 128
F = L // P  # 64
N_CORES = 8

_f32 = mybir.dt.float32
_i32 = mybir.dt.int32
_Alu = mybir.AluOpType
_Act = mybir.ActivationFunctionType

_built = None


def _build_v3(tail="none"):
    nc = bacc.Bacc(
        "TRN2", debug=False, target_bir_lowering=False, num_devices=N_CORES
    )
    # cols 0:F = pred_Y (f32), cols F:2F = true_Y (int32 bitcast to f32)
    xin_d = nc.dram_tensor("xin", [P, 2 * F], _f32, kind="ExternalInput")
    out_d = nc.dram_tensor("out", [1, 1], _f32, kind="ExternalOutput")

    with (
        nc.sbuf_tensor("xt", [P, 2 * F], _f32) as xt,
        nc.sbuf_tensor("p", [P, F], _f32) as p,
        nc.sbuf_tensor("m", [P, F], _f32) as m,
        nc.sbuf_tensor("mp", [P, F], _f32) as mp,
        nc.sbuf_tensor("m0p", [P, F], _f32) as m0p,
        nc.sbuf_tensor("sc1", [P, F], _f32) as sc1,
        nc.sbuf_tensor("sc2", [P, F], _f32) as sc2,
        nc.sbuf_tensor("stats", [P, 8], _f32) as stats,
        nc.sbuf_tensor("ones", [P, 1], _f32) as ones,
        nc.sbuf_tensor("w", [1, 32], _f32) as w,
        nc.psum_tensor("acc", [1, 8], _f32) as acc,
        nc.semaphore("s_in") as s_in,
        nc.semaphore("s_act") as s_act,
        nc.semaphore("s_dve") as s_dve,
        nc.semaphore("s_pe") as s_pe,
        nc.semaphore("s_sp") as s_sp,
    ):
        pred_v = xt[:, 0:F]
        true_v = xt[:, F : 2 * F].bitcast(_i32)

        def c(i, j=None):
            return w[0:1, i : (i + 1 if j is None else j)]

        # w cell map:
        #  0 = 1.0 | 1:4 = (0,-2,0) | 4:7 = (L,0,0) | 7:10 = (-1,2,1)
        #  16:21 = rr copy of totals (s1, s2, n1->alpha, gamma, delta)
        #  21 = y | 22:25 = u = (z, 2*beta, n1) | 25 = pair
        #  26:29 = product scratch | 29 = denom | 30 = 1/denom | 31 = loss

        # --- SP: input DMA trigger (hoisted to stream front post-build) ---
        dma_inst = nc.sync.dma_start(xt[:], xin_d[:]).then_inc(s_in, 16)

        # --- DVE stream (s_dve counts every DVE instruction; the engine may
        # overlap queued ops, so every intra-DVE data dependency carries an
        # explicit wait on the producer's cumulative s_dve count) ---
        ndve = 0

        def dv(inst, after=0):
            nonlocal ndve
            ndve += 1
            if after:
                inst._wait_ge(s_dve, after)
            inst.then_inc(s_dve, 1)
            return ndve

        # disjoint single-cell const memsets (no WAW overlaps)
        dv(nc.vector.memset(c(0), 1.0))
        dv(nc.vector.memset(c(1), 0.0))
        dv(nc.vector.memset(c(2), -2.0))
        dv(nc.vector.memset(c(3), 0.0))
        dv(nc.vector.memset(c(4), float(L)))
        dv(nc.vector.memset(c(5), 0.0))
        dv(nc.vector.memset(c(6), 0.0))
        dv(nc.vector.memset(c(7), -1.0))
        dv(nc.vector.memset(c(8), 2.0))
        dv(nc.vector.memset(c(9), 1.0))
        dv(nc.vector.memset(ones[:], 1.0))

        # stats cols: 0=s1, 1=s2, 2=n1, 3=gamma, 4=delta
        i_m = dv(nc.vector.tensor_copy(m[:], true_v)._wait_ge(s_in, 16))
        dv(
            nc.vector.tensor_reduce(
                stats[:, 2:3], m[:], axis=mybir.AxisListType.X, op=_Alu.add
            ),
            after=i_m,
        )
        # mp needs both p (ACT) and m (DVE): hardware allows one wait per
        # instruction, so park the DVE sequencer on s_act first.
        nc.vector.wait_ge(s_act, 1)
        i_mp = dv(
            nc.vector.scalar_tensor_tensor(
                out=mp[:], in0=m[:], scalar=1.0, in1=p[:],
                op0=_Alu.mult, op1=_Alu.mult, accum_out=stats[:, 0:1],
            ),
            after=i_m,
        )
        i_m0p = dv(
            nc.vector.scalar_tensor_tensor(
                out=m0p[:], in0=mp[:], scalar=-1.0, in1=p[:],
                op0=_Alu.mult, op1=_Alu.add, accum_out=stats[:, 3:4],
            ),
            after=i_mp,
        )
        dv(
            nc.vector.scalar_tensor_tensor(
                out=sc1[:], in0=mp[:], scalar=1.0, in1=p[:],
                op0=_Alu.mult, op1=_Alu.mult, accum_out=stats[:, 1:2],
            ),
            after=i_mp,
        )
        dv(
            nc.vector.scalar_tensor_tensor(
                out=sc2[:], in0=m0p[:], scalar=1.0, in1=p[:],
                op0=_Alu.mult, op1=_Alu.mult, accum_out=stats[:, 4:5],
            ),
            after=i_m0p,
        )
        n_stats = ndve  # PE waits for this many DVE increments

        # --- epilogue on DVE ---
        # rr = totals row (s1, s2, n1, gamma, delta) -> w[16:21]
        i_rr = dv(nc.vector.tensor_copy(c(16, 21), acc[0:1, 0:5])._wait_ge(s_pe, 1))
        # y = s2 - 2*s1
        i_y = dv(
            nc.vector.scalar_tensor_tensor(
                out=c(21), in0=c(16), scalar=-2.0, in1=c(17),
                op0=_Alu.mult, op1=_Alu.add,
            ),
            after=i_rr,
        )
        # alpha = y + n1, overwriting the n1 cell so V = w[18:21] = (alpha, gamma, delta)
        i_v = dv(nc.vector.tensor_add(c(18), c(21), c(18)), after=i_y)
        # u = (0,-2,0)*s1 + (L,0,0) = (L, -2*s1, 0)   [s1 read from PSUM]
        i_u1 = dv(
            nc.vector.scalar_tensor_tensor(
                out=c(22, 25), in0=c(1, 4), scalar=acc[0:1, 0:1], in1=c(4, 7),
                op0=_Alu.mult, op1=_Alu.add,
            )._wait_ge(s_pe, 1)
        )
        # u = (-1,2,1)*n1 + u = (L-n1, 2*(n1-s1), n1)  [n1 read from PSUM]
        i_u = dv(
            nc.vector.scalar_tensor_tensor(
                out=c(22, 25), in0=c(7, 10), scalar=acc[0:1, 2:3], in1=c(22, 25),
                op0=_Alu.mult, op1=_Alu.add,
            ),
            after=i_u1,
        )
        # denom = max(z*n1, 1)  (z, n1 read from u)
        i_den = dv(
            nc.vector.scalar_tensor_tensor(
                out=c(29), in0=c(22), scalar=c(24), in1=c(0),
                op0=_Alu.mult, op1=_Alu.max,
            ),
            after=i_u,
        )
        # pair = sum(u * V)   (i_u > i_v, so waiting on i_u covers both)
        i_pair = dv(
            nc.vector.scalar_tensor_tensor(
                out=c(26, 29), in0=c(22, 25), scalar=1.0, in1=c(18, 21),
                op0=_Alu.mult, op1=_Alu.mult, accum_out=c(25),
            ),
            after=max(i_u, i_v),
        )
        i_rec = dv(nc.vector.reciprocal(c(30), c(29)), after=i_den)
        dv(nc.vector.tensor_mul(c(31), c(25), c(30)), after=max(i_rec, i_pair))
        n_all = ndve

        # --- ACT: sigmoid ---
        nc.scalar.activation(p[:], pred_v, _Act.Sigmoid)._wait_ge(
            s_in, 16
        ).then_inc(s_act, 1)

        # --- PE: partition-axis reduction of the five stat columns ---
        nc.tensor.matmul(
            acc[0:1, 0:5], ones[:], stats[:, 0:5], start=True, stop=True
        )._wait_ge(s_dve, n_stats).then_inc(s_pe, 1)

        # --- SP: 4-byte result via register store (skips the HWDGE path) ---
        reg = nc.sync.alloc_register()
        nc.sync.reg_load(reg, c(31).bitcast(_i32))._wait_ge(s_dve, n_all)
        st = nc.sync.store(out_d[0:1, 0:1].bitcast(_i32), reg)

        if tail == "pool":
            st.then_inc(s_sp, 1)
            sem_range = range(s_in.num, s_sp.num + 1)
            nc.gpsimd.wait_ge(s_sp, 1)
            nc.gpsimd.dma_reset(sem_range)
            nc.gpsimd.sem_clear(sem_range)

    # Hoist the input-DMA trigger in front of the framework preamble barrier
    # (but after SP's TPB-base register setup): it has no dependencies (fresh
    # sems, private SBUF dest), so SP fires it at t~300ns instead of ~670ns
    # and the preamble barrier hides under the DMA latency.
    entry = nc.main_func.blocks[0]
    raw = dma_inst.ins
    insts = entry.instructions
    insts.remove(raw)
    sp_drain = next(
        i
        for i, inst in enumerate(insts)
        if isinstance(inst, mybir.InstDrain) and inst.engine == mybir.EngineType.SP
    )
    insts.insert(sp_drain, raw)

    nc.compile()
    return nc


_build = _build_v3


def _pack(pred_Y, true_Y):
    xin = np.empty((P, 2 * F), dtype=np.float32)
    xin[:, 0:F] = np.ascontiguousarray(pred_Y, dtype=np.float32).reshape(P, F)
    xin[:, F : 2 * F] = (
        np.ascontiguousarray(true_Y, dtype=np.int32).reshape(P, F).view(np.float32)
    )
    return xin


def _run(pred_Y, true_Y, **hw_kwargs):
    global _built
    if _built is None:
        _built = _build()
    in_map = {"xin": _pack(pred_Y, true_Y)}
    res = run_bass_kernel_spmd(
        _built, [in_map] * N_CORES, list(range(N_CORES)), **hw_kwargs
    )
    out = np.asarray(res.results[0]["out"], dtype=np.float32).reshape(())
    return out, res


def kernel(pred_Y, true_Y):
    out, _ = _run(pred_Y, true_Y)
    return out


# revision 12
# speedup vs baseline: 1.3435x; 1.0776x over previous
"""Trainium2 Bass kernel for nn_DIYloss_1709396984424.

Loss: for binary labels, mean over (one, zero) pairs of (1 + p[l] - p[k])^2
where p = sigmoid(pred_Y). With q = 1 - p, each pair term is
(q_k + p_l)^2, so the L^2 sum has the closed form

    pair_sum = n0*alpha + 2*beta*gamma + n1*delta
      n1 = sum(m),        n0 = L - n1        (m = one-mask)
      s1 = sum(m*p),      s2 = sum(m*p^2)
      gamma = sum((1-m)*p),  delta = sum((1-m)*p^2)
      alpha = sum(m*q^2) = n1 - 2*s1 + s2,  beta = sum(m*q) = n1 - s1

    loss = pair_sum / max(n1*n0, 1)

Each of the 8 cores receives the full (replicated) input and computes the
full scalar on-device; core 0's output is returned. The two inputs are
packed host-side into one [128,128] f32 buffer (int32 labels bitcast into
the second half) so a single DMA brings everything in.

Schedule (per core): the SP DMA trigger is hoisted in front of the
framework preamble barrier so the ~640ns preamble (const-AP memsets +
all-engine barrier) hides entirely under the ~2.4us DMA latency. ACT does
the sigmoid; DVE produces the five masked row-sum columns with fused
accum_out; one tiny PE matmul reduces the partition axis; the epilogue is
8 small DVE ops (the 3-term pair_sum is one fused multiply+row-accum over
a 3-lane vector) plus the denominator on the Pool engine. The 4-byte
result goes out via SP register store (no DMA). Tail: SP's store bumps a
semaphore and Pool alone does dma_reset + sem_clear — no all-engine
barrier waves.
"""

import numpy as np

try:
    import concourse.bass as bass  # noqa: F401
except ImportError:  # pragma: no cover - grading env should have it on path
    import sys

    sys.path.insert(0, "/opt/trn_rl_repo")
    import concourse.bass as bass  # noqa: F401

from concourse import bacc, mybir
from concourse.bass_utils import run_bass_kernel_spmd

L = 8192
P = 128
F = L // P  # 64
N_CORES = 8

_f32 = mybir.dt.float32
_i32 = mybir.dt.int32
_Alu = mybir.AluOpType
_Act = mybir.ActivationFunctionType

_built = None


def _build_v3(tail="none"):
    nc = bacc.Bacc(
        "TRN2", debug=False, target_bir_lowering=False, num_devices=N_CORES
    )
    # cols 0:F = pred_Y (f32), cols F:2F = true_Y (int32 bitcast to f32)
    xin_d = nc.dram_tensor("xin", [P, 2 * F], _f32, kind="ExternalInput")
    out_d = nc.dram_tensor("out", [1, 1], _f32, kind="ExternalOutput")

    with (
        nc.sbuf_tensor("xt", [P, 2 * F], _f32) as xt,
        nc.sbuf_tensor("p", [P, F], _f32) as p,
        nc.sbuf_tensor("m", [P, F], _f32) as m,
        nc.sbuf_tensor("mp", [P, F], _f32) as mp,
        nc.sbuf_tensor("m0p", [P, F], _f32) as m0p,
        nc.sbuf_tensor("sc1", [P, F], _f32) as sc1,
        nc.sbuf_tensor("sc2", [P, F], _f32) as sc2,
        nc.sbuf_tensor("stats", [P, 8], _f32) as stats,
        nc.sbuf_tensor("ones", [P, 1], _f32) as ones,
        nc.sbuf_tensor("w", [1, 32], _f32) as w,
        nc.psum_tensor("acc", [1, 8], _f32) as acc,
        nc.semaphore("s_in") as s_in,
        nc.semaphore("s_act") as s_act,
        nc.semaphore("s_dve") as s_dve,
        nc.semaphore("s_pe") as s_pe,
        nc.semaphore("s_sp") as s_sp,
    ):
        pred_v = xt[:, 0:F]
        true_v = xt[:, F : 2 * F].bitcast(_i32)

        def c(i, j=None):
            return w[0:1, i : (i + 1 if j is None else j)]

        # w cell map:
        #  0 = 1.0 | 1:4 = (0,-2,0) | 4:7 = (L,0,0) | 7:10 = (-1,2,1)
        #  16:21 = rr copy of totals (s1, s2, n1->alpha, gamma, delta)
        #  21 = y | 22:25 = u = (z, 2*beta, n1) | 25 = pair
        #  26:29 = product scratch | 29 = denom | 30 = 1/denom | 31 = loss

        # --- SP: input DMA trigger (hoisted to stream front post-build) ---
        dma_inst = nc.sync.dma_start(xt[:], xin_d[:]).then_inc(s_in, 16)

        # --- DVE stream (s_dve counts every DVE instruction; the engine may
        # overlap queued ops, so every intra-DVE data dependency carries an
        # explicit wait on the producer's cumulative s_dve count) ---
        ndve = 0

        def dv(inst, after=0):
            nonlocal ndve
            ndve += 1
            if after:
                inst._wait_ge(s_dve, after)
            inst.then_inc(s_dve, 1)
            return ndve

        # disjoint single-cell const memsets (no WAW overlaps)
        dv(nc.vector.memset(c(0), 1.0))
        dv(nc.vector.memset(c(1), 0.0))
        dv(nc.vector.memset(c(2), -2.0))
        dv(nc.vector.memset(c(3), 0.0))
        dv(nc.vector.memset(c(4), float(L)))
        dv(nc.vector.memset(c(5), 0.0))
        dv(nc.vector.memset(c(6), 0.0))
        dv(nc.vector.memset(c(7), -1.0))
        dv(nc.vector.memset(c(8), 2.0))
        dv(nc.vector.memset(c(9), 1.0))
        dv(nc.vector.memset(ones[:], 1.0))

        # stats cols: 0=s1, 1=s2, 2=n1, 3=gamma, 4=delta
        i_m = dv(nc.vector.tensor_copy(m[:], true_v)._wait_ge(s_in, 16))
        dv(
            nc.vector.tensor_reduce(
                stats[:, 2:3], m[:], axis=mybir.AxisListType.X, op=_Alu.add
            ),
            after=i_m,
        )
        # mp needs both p (ACT) and m (DVE): hardware allows one wait per
        # instruction, so park the DVE sequencer on s_act first.
        nc.vector.wait_ge(s_act, 1)
        i_mp = dv(
            nc.vector.scalar_tensor_tensor(
                out=mp[:], in0=m[:], scalar=1.0, in1=p[:],
                op0=_Alu.mult, op1=_Alu.mult, accum_out=stats[:, 0:1],
            ),
            after=i_m,
        )
        i_m0p = dv(
            nc.vector.scalar_tensor_tensor(
                out=m0p[:], in0=mp[:], scalar=-1.0, in1=p[:],
                op0=_Alu.mult, op1=_Alu.add, accum_out=stats[:, 3:4],
            ),
            after=i_mp,
        )
        dv(
            nc.vector.scalar_tensor_tensor(
                out=sc1[:], in0=mp[:], scalar=1.0, in1=p[:],
                op0=_Alu.mult, op1=_Alu.mult, accum_out=stats[:, 1:2],
            ),
            after=i_mp,
        )
        dv(
            nc.vector.scalar_tensor_tensor(
                out=sc2[:], in0=m0p[:], scalar=1.0, in1=p[:],
                op0=_Alu.mult, op1=_Alu.mult, accum_out=stats[:, 4:5],
            ),
            after=i_m0p,
        )
        n_stats = ndve  # PE waits for this many DVE increments

        # --- epilogue on DVE ---
        # rr = totals row (s1, s2, n1, gamma, delta) -> w[16:21]
        i_rr = dv(nc.vector.tensor_copy(c(16, 21), acc[0:1, 0:5])._wait_ge(s_pe, 1))
        # y = s2 - 2*s1
        i_y = dv(
            nc.vector.scalar_tensor_tensor(
                out=c(21), in0=c(16), scalar=-2.0, in1=c(17),
                op0=_Alu.mult, op1=_Alu.add,
            ),
            after=i_rr,
        )
        # alpha = y + n1, overwriting the n1 cell so V = w[18:21] = (alpha, gamma, delta)
        i_v = dv(nc.vector.tensor_add(c(18), c(21), c(18)), after=i_y)
        # u = (0,-2,0)*s1 + (L,0,0) = (L, -2*s1, 0)   [s1 read from PSUM]
        i_u1 = dv(
            nc.vector.scalar_tensor_tensor(
                out=c(22, 25), in0=c(1, 4), scalar=acc[0:1, 0:1], in1=c(4, 7),
                op0=_Alu.mult, op1=_Alu.add,
            )._wait_ge(s_pe, 1)
        )
        # u = (-1,2,1)*n1 + u = (L-n1, 2*(n1-s1), n1)  [n1 read from PSUM]
        i_u = dv(
            nc.vector.scalar_tensor_tensor(
                out=c(22, 25), in0=c(7, 10), scalar=acc[0:1, 2:3], in1=c(22, 25),
                op0=_Alu.mult, op1=_Alu.add,
            ),
            after=i_u1,
        )
        # denom = max(z*n1, 1)  (z, n1 read from u)
        i_den = dv(
            nc.vector.scalar_tensor_tensor(
                out=c(29), in0=c(22), scalar=c(24), in1=c(0),
                op0=_Alu.mult, op1=_Alu.max,
            ),
            after=i_u,
        )
        # pair = sum(u * V)   (i_u > i_v, so waiting on i_u covers both)
        i_pair = dv(
            nc.vector.scalar_tensor_tensor(
                out=c(26, 29), in0=c(22, 25), scalar=1.0, in1=c(18, 21),
                op0=_Alu.mult, op1=_Alu.mult, accum_out=c(25),
            ),
            after=max(i_u, i_v),
        )
        i_rec = dv(nc.vector.reciprocal(c(30), c(29)), after=i_den)
        dv(nc.vector.tensor_mul(c(31), c(25), c(30)), after=max(i_rec, i_pair))
        n_all = ndve

        # --- ACT: sigmoid ---
        nc.scalar.activation(p[:], pred_v, _Act.Sigmoid)._wait_ge(
            s_in, 16
        ).then_inc(s_act, 1)

        # --- PE: partition-axis reduction of the five stat columns ---
        nc.tensor.matmul(
            acc[0:1, 0:5], ones[:], stats[:, 0:5], start=True, stop=True
        )._wait_ge(s_dve, n_stats).then_inc(s_pe, 1)

        # --- SP: 4-byte result via register store (skips the HWDGE path) ---
        reg = nc.sync.alloc_register()
        nc.sync.reg_load(reg, c(31).bitcast(_i32))._wait_ge(s_dve, n_all)
        st = nc.sync.store(out_d[0:1, 0:1].bitcast(_i32), reg)

        if tail == "pool":
            st.then_inc(s_sp, 1)
            sem_range = range(s_in.num, s_sp.num + 1)
            nc.gpsimd.wait_ge(s_sp, 1)
            nc.gpsimd.dma_reset(sem_range)
            nc.gpsimd.sem_clear(sem_range)

    # Hoist the input-DMA trigger in front of the framework preamble barrier
    # (but after SP's TPB-base register setup): it has no dependencies (fresh
    # sems, private SBUF dest), so SP fires it at t~300ns instead of ~670ns
    # and the preamble barrier hides under the DMA latency.
    entry = nc.main_func.blocks[0]
    raw = dma_inst.ins
    insts = entry.instructions
    insts.remove(raw)
    sp_drain = next(
        i
        for i, inst in enumerate(insts)
        if isinstance(inst, mybir.InstDrain) and inst.engine == mybir.EngineType.SP
    )
    insts.insert(sp_drain, raw)

    nc.compile()
    return nc


def _build_v4(tail="none"):
    """Depth-optimized schedule.

    Stats phase is 2 sem-hops deep after p (mp/p2/reduce depend only on p;
    s2 = m*p2), the totals row is (s1, s2, n1, Tp, Tp2), and the epilogue
    rebuilds V = (alpha, gamma, delta) = (n1,Tp,Tp2) + s2*(1,0,-1) +
    s1*(-2,-1,0) and u = (z, 2*beta, n1) in two fused pointer-scalar hops
    each, reading the totals straight from PSUM (the scalar-pointer operand
    is exempt from the one-PSUM-operand rule).
    """
    nc = bacc.Bacc(
        "TRN2", debug=False, target_bir_lowering=False, num_devices=N_CORES
    )
    xin_d = nc.dram_tensor("xin", [P, 2 * F], _f32, kind="ExternalInput")
    out_d = nc.dram_tensor("out", [1, 1], _f32, kind="ExternalOutput")

    with (
        nc.sbuf_tensor("xt", [P, 2 * F], _f32) as xt,
        nc.sbuf_tensor("p", [P, F], _f32) as p,
        nc.sbuf_tensor("m", [P, F], _f32) as m,
        nc.sbuf_tensor("mp", [P, F], _f32) as mp,
        nc.sbuf_tensor("p2", [P, F], _f32) as p2,
        nc.sbuf_tensor("sc1", [P, F], _f32) as sc1,
        nc.sbuf_tensor("stats", [P, 8], _f32) as stats,
        nc.sbuf_tensor("ones", [P, 1], _f32) as ones,
        nc.sbuf_tensor("w", [1, 32], _f32) as w,
        nc.psum_tensor("acc", [1, 8], _f32) as acc,
        nc.semaphore("s_in") as s_in,
        nc.semaphore("s_act") as s_act,
        nc.semaphore("s_dve") as s_dve,
        nc.semaphore("s_pe") as s_pe,
    ):
        pred_v = xt[:, 0:F]
        true_v = xt[:, F : 2 * F].bitcast(_i32)

        def c(i, j=None):
            return w[0:1, i : (i + 1 if j is None else j)]

        # w cells: 0=1.0 | 1:4=cB=(1,0,-1) | 4:7=cL=(L,0,0) | 7:10=cU2=(-1,2,1)
        # 10:13=cA=(-2,-1,0) | 13:16=cU1=(0,-2,0)
        # 16:19=va->V | 19:22=ua->u | 22=z | 23=den | 24=pair | 25=rec
        # 26=loss | 27:30=pair product scratch

        dma_inst = nc.sync.dma_start(xt[:], xin_d[:]).then_inc(s_in, 16)

        ndve = 0

        def dv(inst, after=0):
            nonlocal ndve
            ndve += 1
            if after:
                inst._wait_ge(s_dve, after)
            inst.then_inc(s_dve, 1)
            return ndve

        i_zero = dv(nc.vector.memset(c(0, 16), 0.0))
        for cell, val in [
            (0, 1.0), (1, 1.0), (3, -1.0), (4, float(L)), (7, -1.0),
            (8, 2.0), (9, 1.0), (10, -2.0), (11, -1.0), (14, -2.0),
        ]:
            dv(nc.vector.memset(c(cell), val), after=i_zero)
        dv(nc.vector.memset(ones[:], 1.0))

        # stats cols: 0=s1, 1=s2, 2=n1, 3=Tp, 4=Tp2
        # int32 -> f32 cast + row-sum (the HW tensor-scalar reduce rejects
        # int inputs, so this stays a copy + reduce; both run before p lands)
        i_m = dv(nc.vector.tensor_copy(m[:], true_v)._wait_ge(s_in, 16))
        dv(
            nc.vector.tensor_reduce(
                stats[:, 2:3], m[:], axis=mybir.AxisListType.X, op=_Alu.add
            ),
            after=i_m,
        )
        # park the sequencer until p is ready; the three ops below depend
        # only on p (and m, already ordered) — depth 1 after p
        nc.vector.wait_ge(s_act, 1)
        dv(
            nc.vector.scalar_tensor_tensor(
                out=mp[:], in0=m[:], scalar=1.0, in1=p[:],
                op0=_Alu.mult, op1=_Alu.mult, accum_out=stats[:, 0:1],
            ),
            after=i_m,
        )
        i_p2 = dv(
            nc.vector.scalar_tensor_tensor(
                out=p2[:], in0=p[:], scalar=1.0, in1=p[:],
                op0=_Alu.mult, op1=_Alu.mult, accum_out=stats[:, 4:5],
            )
        )
        dv(
            nc.vector.tensor_reduce(
                stats[:, 3:4], p[:], axis=mybir.AxisListType.X, op=_Alu.add
            )
        )
        # depth 2: s2 = sum(m * p^2)
        dv(
            nc.vector.scalar_tensor_tensor(
                out=sc1[:], in0=m[:], scalar=1.0, in1=p2[:],
                op0=_Alu.mult, op1=_Alu.mult, accum_out=stats[:, 1:2],
            ),
            after=i_p2,
        )
        n_stats = ndve

        # --- epilogue: 4 sem-hops from totals to loss ---
        # hop 1 (all gated on s_pe only)
        i_va = dv(
            nc.vector.scalar_tensor_tensor(
                out=c(16, 19), in0=c(1, 4), scalar=acc[0:1, 1:2],
                in1=acc[0:1, 2:5], op0=_Alu.mult, op1=_Alu.add,
            )._wait_ge(s_pe, 1)
        )  # va = s2*(1,0,-1) + (n1,Tp,Tp2)
        i_ua = dv(
            nc.vector.scalar_tensor_tensor(
                out=c(19, 22), in0=c(13, 16), scalar=acc[0:1, 0:1],
                in1=c(4, 7), op0=_Alu.mult, op1=_Alu.add,
            )._wait_ge(s_pe, 1)
        )  # ua = s1*(0,-2,0) + (L,0,0)
        i_z = dv(
            nc.vector.scalar_tensor_tensor(
                out=c(22), in0=acc[0:1, 2:3], scalar=-1.0, in1=c(4),
                op0=_Alu.mult, op1=_Alu.add,
            )._wait_ge(s_pe, 1)
        )  # z = L - n1
        # hop 2
        i_V = dv(
            nc.vector.scalar_tensor_tensor(
                out=c(16, 19), in0=c(10, 13), scalar=acc[0:1, 0:1],
                in1=c(16, 19), op0=_Alu.mult, op1=_Alu.add,
            ),
            after=i_va,
        )  # V = s1*(-2,-1,0) + va = (alpha, gamma, delta)
        i_u = dv(
            nc.vector.scalar_tensor_tensor(
                out=c(19, 22), in0=c(7, 10), scalar=acc[0:1, 2:3],
                in1=c(19, 22), op0=_Alu.mult, op1=_Alu.add,
            ),
            after=i_ua,
        )  # u = n1*(-1,2,1) + ua = (z, 2*beta, n1)
        i_den = dv(
            nc.vector.scalar_tensor_tensor(
                out=c(23), in0=c(22), scalar=acc[0:1, 2:3], in1=c(0),
                op0=_Alu.mult, op1=_Alu.max,
            ),
            after=i_z,
        )  # den = max(z*n1, 1)
        # hop 3
        i_pair = dv(
            nc.vector.scalar_tensor_tensor(
                out=c(27, 30), in0=c(19, 22), scalar=1.0, in1=c(16, 19),
                op0=_Alu.mult, op1=_Alu.mult, accum_out=c(24),
            ),
            after=i_u,
        )  # pair = sum(u * V)
        i_rec = dv(nc.vector.reciprocal(c(25), c(23)), after=i_den)
        # hop 4
        dv(nc.vector.tensor_mul(c(26), c(24), c(25)), after=i_rec)
        n_all = ndve

        nc.scalar.activation(p[:], pred_v, _Act.Sigmoid)._wait_ge(
            s_in, 16
        ).then_inc(s_act, 1)

        nc.tensor.matmul(
            acc[0:1, 0:5], ones[:], stats[:, 0:5], start=True, stop=True
        )._wait_ge(s_dve, n_stats).then_inc(s_pe, 1)

        reg = nc.sync.alloc_register()
        nc.sync.reg_load(reg, c(26).bitcast(_i32))._wait_ge(s_dve, n_all)
        nc.sync.store(out_d[0:1, 0:1].bitcast(_i32), reg)
        assert tail == "none"

    entry = nc.main_func.blocks[0]
    raw = dma_inst.ins
    insts = entry.instructions
    insts.remove(raw)
    sp_drain = next(
        i
        for i, inst in enumerate(insts)
        if isinstance(inst, mybir.InstDrain) and inst.engine == mybir.EngineType.SP
    )
    insts.insert(sp_drain, raw)

    nc.compile()
    return nc


_build = _build_v4


def _pack(pred_Y, true_Y):
    xin = np.empty((P, 2 * F), dtype=np.float32)
    xin[:, 0:F] = np.ascontiguousarray(pred_Y, dtype=np.float32).reshape(P, F)
    xin[:, F : 2 * F] = (
        np.ascontiguousarray(true_Y, dtype=np.int32).reshape(P, F).view(np.float32)
    )
    return xin


def _run(pred_Y, true_Y, **hw_kwargs):
    global _built
    if _built is None:
        _built = _build()
    in_map = {"xin": _pack(pred_Y, true_Y)}
    res = run_bass_kernel_spmd(
        _built, [in_map] * N_CORES, list(range(N_CORES)), **hw_kwargs
    )
    out = np.asarray(res.results[0]["out"], dtype=np.float32).reshape(())
    return out, res


def kernel(pred_Y, true_Y):
    out, _ = _run(pred_Y, true_Y)
    return out


# revision 18
# speedup vs baseline: 1.3827x; 1.0292x over previous
"""Trainium2 Bass kernel for nn_DIYloss_1709396984424.

Loss: for binary labels, mean over (one, zero) pairs of (1 + p[l] - p[k])^2
where p = sigmoid(pred_Y). With q = 1 - p, each pair term is
(q_k + p_l)^2, so the L^2 sum has the closed form

    pair_sum = n0*alpha + 2*beta*gamma + n1*delta
      n1 = sum(m),        n0 = L - n1        (m = one-mask)
      s1 = sum(m*p),      s2 = sum(m*p^2)
      gamma = sum((1-m)*p),  delta = sum((1-m)*p^2)
      alpha = sum(m*q^2) = n1 - 2*s1 + s2,  beta = sum(m*q) = n1 - s1

    loss = pair_sum / max(n1*n0, 1)

Each of the 8 cores receives the full (replicated) input and computes the
full scalar on-device; core 0's output is returned. The two inputs are
packed host-side into one [128,128] f32 buffer (int32 labels bitcast into
the second half) so a single DMA brings everything in.

Schedule (per core): the SP DMA trigger is hoisted in front of the
framework preamble barrier so the ~640ns preamble (const-AP memsets +
all-engine barrier) hides entirely under the ~2.4us DMA latency. ACT does
the sigmoid; DVE produces the five masked row-sum columns with fused
accum_out; one tiny PE matmul reduces the partition axis; the epilogue is
9 small DVE ops in 4 semaphore hops (the 3-term pair_sum is one fused
multiply+row-accum over a 3-lane vector reading totals from PSUM via
pointer scalars). The 4-byte result goes out via SP register store (no
DMA). No reset tail: each execution's waits are satisfied monotonically,
and re-execution with the same NEFF state stays deterministic (verified
by the double-call check in test.py).
"""

import numpy as np

try:
    import concourse.bass as bass  # noqa: F401
except ImportError:  # pragma: no cover - grading env should have it on path
    import sys

    sys.path.insert(0, "/opt/trn_rl_repo")
    import concourse.bass as bass  # noqa: F401

from concourse import bacc, mybir
from concourse.bass_utils import run_bass_kernel_spmd

L = 8192
P = 128
F = L // P  # 64
N_CORES = 8

_f32 = mybir.dt.float32
_i32 = mybir.dt.int32
_Alu = mybir.AluOpType
_Act = mybir.ActivationFunctionType

_built = None


def _build_v4(tail="none"):
    """Depth-optimized schedule.

    Stats phase is 2 sem-hops deep after p (mp/p2/reduce depend only on p;
    s2 = m*p2), the totals row is (s1, s2, n1, Tp, Tp2), and the epilogue
    rebuilds V = (alpha, gamma, delta) = (n1,Tp,Tp2) + s2*(1,0,-1) +
    s1*(-2,-1,0) and u = (z, 2*beta, n1) in two fused pointer-scalar hops
    each, reading the totals straight from PSUM (the scalar-pointer operand
    is exempt from the one-PSUM-operand rule).
    """
    nc = bacc.Bacc(
        "TRN2", debug=False, target_bir_lowering=False, num_devices=N_CORES
    )
    xin_d = nc.dram_tensor("xin", [P, 2 * F], _f32, kind="ExternalInput")
    out_d = nc.dram_tensor("out", [1, 1], _f32, kind="ExternalOutput")

    with (
        nc.sbuf_tensor("xt", [P, 2 * F], _f32) as xt,
        nc.sbuf_tensor("p", [P, F], _f32) as p,
        nc.sbuf_tensor("m", [P, F], _f32) as m,
        nc.sbuf_tensor("mp", [P, F], _f32) as mp,
        nc.sbuf_tensor("p2", [P, F], _f32) as p2,
        nc.sbuf_tensor("sc1", [P, F], _f32) as sc1,
        nc.sbuf_tensor("stats", [P, 8], _f32) as stats,
        nc.sbuf_tensor("ones", [P, 1], _f32) as ones,
        nc.sbuf_tensor("w", [1, 32], _f32) as w,
        nc.psum_tensor("acc", [1, 8], _f32) as acc,
        nc.semaphore("s_in") as s_in,
        nc.semaphore("s_act") as s_act,
        nc.semaphore("s_dve") as s_dve,
        nc.semaphore("s_pe") as s_pe,
    ):
        pred_v = xt[:, 0:F]
        true_v = xt[:, F : 2 * F].bitcast(_i32)

        def c(i, j=None):
            return w[0:1, i : (i + 1 if j is None else j)]

        # w cells: 0=1.0 | 1:4=cB=(1,0,-1) | 4:7=cL=(L,0,0) | 7:10=cU2=(-1,2,1)
        # 10:13=cA=(-2,-1,0) | 13:16=cU1=(0,-2,0)
        # 16:19=va->V | 19:22=ua->u | 22=z | 23=den | 24=pair | 25=rec
        # 26=loss | 27:30=pair product scratch

        dma_inst = nc.sync.dma_start(xt[:], xin_d[:]).then_inc(s_in, 16)

        ndve = 0

        def dv(inst, after=0):
            nonlocal ndve
            ndve += 1
            if after:
                inst._wait_ge(s_dve, after)
            inst.then_inc(s_dve, 1)
            return ndve

        i_zero = dv(nc.vector.memset(c(0, 16), 0.0))
        for cell, val in [
            (0, 1.0), (1, 1.0), (3, -1.0), (4, float(L)), (7, -1.0),
            (8, 2.0), (9, 1.0), (10, -2.0), (11, -1.0), (14, -2.0),
        ]:
            dv(nc.vector.memset(c(cell), val), after=i_zero)
        dv(nc.vector.memset(ones[:], 1.0))

        # stats cols: 0=s1, 1=s2, 2=n1, 3=Tp, 4=Tp2
        # int32 -> f32 cast + row-sum (the HW tensor-scalar reduce rejects
        # int inputs, so this stays a copy + reduce; both run before p lands)
        i_m = dv(nc.vector.tensor_copy(m[:], true_v)._wait_ge(s_in, 16))
        dv(
            nc.vector.tensor_reduce(
                stats[:, 2:3], m[:], axis=mybir.AxisListType.X, op=_Alu.add
            ),
            after=i_m,
        )
        # park the sequencer until p is ready; mp and the Tp reduce depend
        # only on p (and m, already ordered) — depth 1 after p
        nc.vector.wait_ge(s_act, 1)
        i_mp = dv(
            nc.vector.scalar_tensor_tensor(
                out=mp[:], in0=m[:], scalar=1.0, in1=p[:],
                op0=_Alu.mult, op1=_Alu.mult, accum_out=stats[:, 0:1],
            ),
            after=i_m,
        )
        dv(
            nc.vector.tensor_reduce(
                stats[:, 3:4], p[:], axis=mybir.AxisListType.X, op=_Alu.add
            )
        )
        # depth 2: s2 = sum((m*p) * p)
        dv(
            nc.vector.scalar_tensor_tensor(
                out=sc1[:], in0=mp[:], scalar=1.0, in1=p[:],
                op0=_Alu.mult, op1=_Alu.mult, accum_out=stats[:, 1:2],
            ),
            after=i_mp,
        )
        # bridge: fold ACT's Square completion (s_act=2) into the s_dve count
        # so the matmul's single wait covers every stats producer; keeping all
        # s_dve updates on one engine keeps the count deterministic
        dv(nc.vector.wait_ge(s_act, 2))
        n_stats = ndve

        # --- epilogue: 4 sem-hops from totals to loss ---
        # hop 1 (all gated on s_pe only)
        i_va = dv(
            nc.vector.scalar_tensor_tensor(
                out=c(16, 19), in0=c(1, 4), scalar=acc[0:1, 1:2],
                in1=acc[0:1, 2:5], op0=_Alu.mult, op1=_Alu.add,
            )._wait_ge(s_pe, 1)
        )  # va = s2*(1,0,-1) + (n1,Tp,Tp2)
        i_ua = dv(
            nc.vector.scalar_tensor_tensor(
                out=c(19, 22), in0=c(13, 16), scalar=acc[0:1, 0:1],
                in1=c(4, 7), op0=_Alu.mult, op1=_Alu.add,
            )._wait_ge(s_pe, 1)
        )  # ua = s1*(0,-2,0) + (L,0,0)
        i_z = dv(
            nc.vector.scalar_tensor_tensor(
                out=c(22), in0=acc[0:1, 2:3], scalar=-1.0, in1=c(4),
                op0=_Alu.mult, op1=_Alu.add,
            )._wait_ge(s_pe, 1)
        )  # z = L - n1
        # hop 2
        i_V = dv(
            nc.vector.scalar_tensor_tensor(
                out=c(16, 19), in0=c(10, 13), scalar=acc[0:1, 0:1],
                in1=c(16, 19), op0=_Alu.mult, op1=_Alu.add,
            ),
            after=i_va,
        )  # V = s1*(-2,-1,0) + va = (alpha, gamma, delta)
        i_u = dv(
            nc.vector.scalar_tensor_tensor(
                out=c(19, 22), in0=c(7, 10), scalar=acc[0:1, 2:3],
                in1=c(19, 22), op0=_Alu.mult, op1=_Alu.add,
            ),
            after=i_ua,
        )  # u = n1*(-1,2,1) + ua = (z, 2*beta, n1)
        i_den = dv(
            nc.vector.scalar_tensor_tensor(
                out=c(23), in0=c(22), scalar=acc[0:1, 2:3], in1=c(0),
                op0=_Alu.mult, op1=_Alu.max,
            ),
            after=i_z,
        )  # den = max(z*n1, 1)
        # hop 3
        i_pair = dv(
            nc.vector.scalar_tensor_tensor(
                out=c(27, 30), in0=c(19, 22), scalar=1.0, in1=c(16, 19),
                op0=_Alu.mult, op1=_Alu.mult, accum_out=c(24),
            ),
            after=i_u,
        )  # pair = sum(u * V)
        i_rec = dv(nc.vector.reciprocal(c(25), c(23)), after=i_den)
        # hop 4
        dv(nc.vector.tensor_mul(c(26), c(24), c(25)), after=i_rec)
        n_all = ndve

        nc.scalar.activation(p[:], pred_v, _Act.Sigmoid)._wait_ge(
            s_in, 16
        ).then_inc(s_act, 1)
        # Tp2 = sum(p^2) on ACT (Square shares the sigmoid table set). Its
        # completion is signaled on s_dve: the matmul's single wait below
        # covers both producers because DVE epilogue increments cannot occur
        # until s_pe fires, so s_dve == n_stats + 1 iff all DVE stats ops AND
        # this op are done.
        nc.scalar.wait_ge(s_act, 1)
        nc.scalar.activation(
            p2[:], p[:], _Act.Square, accum_out=stats[:, 4:5]
        ).then_inc(s_act, 1)

        nc.tensor.matmul(
            acc[0:1, 0:5], ones[:], stats[:, 0:5], start=True, stop=True
        )._wait_ge(s_dve, n_stats).then_inc(s_pe, 1)

        reg = nc.sync.alloc_register()
        nc.sync.reg_load(reg, c(26).bitcast(_i32))._wait_ge(s_dve, n_all)
        nc.sync.store(out_d[0:1, 0:1].bitcast(_i32), reg)
        assert tail == "none"

    entry = nc.main_func.blocks[0]
    raw = dma_inst.ins
    insts = entry.instructions
    insts.remove(raw)
    sp_drain = next(
        i
        for i, inst in enumerate(insts)
        if isinstance(inst, mybir.InstDrain) and inst.engine == mybir.EngineType.SP
    )
    insts.insert(sp_drain, raw)

    nc.compile()
    return nc


_build = _build_v4


def _pack(pred_Y, true_Y):
    xin = np.empty((P, 2 * F), dtype=np.float32)
    xin[:, 0:F] = np.ascontiguousarray(pred_Y, dtype=np.float32).reshape(P, F)
    xin[:, F : 2 * F] = (
        np.ascontiguousarray(true_Y, dtype=np.int32).reshape(P, F).view(np.float32)
    )
    return xin


def _run(pred_Y, true_Y, **hw_kwargs):
    global _built
    if _built is None:
        _built = _build()
    in_map = {"xin": _pack(pred_Y, true_Y)}
    res = run_bass_kernel_spmd(
        _built, [in_map] * N_CORES, list(range(N_CORES)), **hw_kwargs
    )
    out = np.asarray(res.results[0]["out"], dtype=np.float32).reshape(())
    return out, res


def kernel(pred_Y, true_Y):
    out, _ = _run(pred_Y, true_Y)
    return out


# revision 19
# speedup vs baseline: 1.3923x; 1.0069x over previous
"""Trainium2 Bass kernel for nn_DIYloss_1709396984424.

Loss: for binary labels, mean over (one, zero) pairs of (1 + p[l] - p[k])^2
where p = sigmoid(pred_Y). With q = 1 - p, each pair term is
(q_k + p_l)^2, so the L^2 sum has the closed form

    pair_sum = n0*alpha + 2*beta*gamma + n1*delta
      n1 = sum(m),        n0 = L - n1        (m = one-mask)
      s1 = sum(m*p),      s2 = sum(m*p^2)
      gamma = sum((1-m)*p),  delta = sum((1-m)*p^2)
      alpha = sum(m*q^2) = n1 - 2*s1 + s2,  beta = sum(m*q) = n1 - s1

    loss = pair_sum / max(n1*n0, 1)

Each of the 8 cores receives the full (replicated) input and computes the
full scalar on-device; core 0's output is returned. The two inputs are
packed host-side into one [128,128] f32 buffer (int32 labels bitcast into
the second half) so a single DMA brings everything in.

Schedule (per core): the SP DMA trigger is hoisted in front of the
framework preamble barrier so the ~640ns preamble (const-AP memsets +
all-engine barrier) hides entirely under the ~2.4us DMA latency. ACT does
the sigmoid; DVE produces the five masked row-sum columns with fused
accum_out; one tiny PE matmul reduces the partition axis; the epilogue is
9 small DVE ops in 4 semaphore hops (the 3-term pair_sum is one fused
multiply+row-accum over a 3-lane vector reading totals from PSUM via
pointer scalars). The 4-byte result goes out via SP register store (no
DMA). No reset tail: each execution's waits are satisfied monotonically,
and re-execution with the same NEFF state stays deterministic (verified
by the double-call check in test.py).
"""

import numpy as np

try:
    import concourse.bass as bass  # noqa: F401
except ImportError:  # pragma: no cover - grading env should have it on path
    import sys

    sys.path.insert(0, "/opt/trn_rl_repo")
    import concourse.bass as bass  # noqa: F401

from concourse import bacc, mybir
from concourse.bass_utils import run_bass_kernel_spmd

L = 8192
P = 128
F = L // P  # 64
N_CORES = 8

_f32 = mybir.dt.float32
_i32 = mybir.dt.int32
_Alu = mybir.AluOpType
_Act = mybir.ActivationFunctionType

_built = None


def _build_v4(tail="none"):
    """Depth-optimized schedule.

    Stats phase is 2 sem-hops deep after p (mp/p2/reduce depend only on p;
    s2 = m*p2), the totals row is (s1, s2, n1, Tp, Tp2), and the epilogue
    rebuilds V = (alpha, gamma, delta) = (n1,Tp,Tp2) + s2*(1,0,-1) +
    s1*(-2,-1,0) and u = (z, 2*beta, n1) in two fused pointer-scalar hops
    each, reading the totals straight from PSUM (the scalar-pointer operand
    is exempt from the one-PSUM-operand rule).
    """
    nc = bacc.Bacc(
        "TRN2", debug=False, target_bir_lowering=False, num_devices=N_CORES
    )
    xin_d = nc.dram_tensor("xin", [P, 2 * F], _f32, kind="ExternalInput")
    out_d = nc.dram_tensor("out", [1, 1], _f32, kind="ExternalOutput")

    with (
        nc.sbuf_tensor("xt", [P, 2 * F], _f32) as xt,
        nc.sbuf_tensor("p", [P, F], _f32) as p,
        nc.sbuf_tensor("m", [P, F], _f32) as m,
        nc.sbuf_tensor("mp", [P, F], _f32) as mp,
        nc.sbuf_tensor("p2", [P, F], _f32) as p2,
        nc.sbuf_tensor("sc1", [P, F], _f32) as sc1,
        nc.sbuf_tensor("stats", [P, 8], _f32) as stats,
        nc.sbuf_tensor("ones", [P, 1], _f32) as ones,
        nc.sbuf_tensor("w", [1, 32], _f32) as w,
        nc.psum_tensor("acc", [1, 8], _f32) as acc,
        nc.semaphore("s_in") as s_in,
        nc.semaphore("s_act") as s_act,
        nc.semaphore("s_dve") as s_dve,
        nc.semaphore("s_pe") as s_pe,
    ):
        pred_v = xt[:, 0:F]
        true_v = xt[:, F : 2 * F].bitcast(_i32)

        def c(i, j=None):
            return w[0:1, i : (i + 1 if j is None else j)]

        # w cells: 0=1.0 | 1:4=cB=(1,0,-1) | 4:7=cL=(L,0,0) | 7:10=cU2=(-1,2,1)
        # 10:13=cA=(-2,-1,0) | 13:16=cU1=(0,-2,0)
        # 16:19=va->V | 19:22=ua->u | 22=z | 23=den | 24=pair | 25=rec
        # 26=loss | 27:30=pair product scratch

        dma_inst = nc.sync.dma_start(xt[:], xin_d[:]).then_inc(s_in, 16)

        ndve = 0

        def dv(inst, after=0):
            nonlocal ndve
            ndve += 1
            if after:
                inst._wait_ge(s_dve, after)
            inst.then_inc(s_dve, 1)
            return ndve

        i_zero = dv(nc.vector.memset(c(0, 16), 0.0))
        for cell, val in [
            (0, 1.0), (1, 1.0), (3, -1.0), (4, float(L)), (7, -1.0),
            (8, 2.0), (9, 1.0), (10, -2.0), (11, -1.0), (14, -2.0),
        ]:
            dv(nc.vector.memset(c(cell), val), after=i_zero)
        dv(nc.vector.memset(ones[:], 1.0))

        # stats cols: 0=s1, 1=s2, 2=n1, 3=Tp, 4=Tp2
        # int32 -> f32 cast + row-sum (the HW tensor-scalar reduce rejects
        # int inputs, so this stays a copy + reduce; both run before p lands)
        i_m = dv(nc.vector.tensor_copy(m[:], true_v)._wait_ge(s_in, 16))
        dv(
            nc.vector.tensor_reduce(
                stats[:, 2:3], m[:], axis=mybir.AxisListType.X, op=_Alu.add
            ),
            after=i_m,
        )
        # park the sequencer until p is ready; mp and the Tp reduce depend
        # only on p (and m, already ordered) — depth 1 after p
        nc.vector.wait_ge(s_act, 1)
        i_mp = dv(
            nc.vector.scalar_tensor_tensor(
                out=mp[:], in0=m[:], scalar=1.0, in1=p[:],
                op0=_Alu.mult, op1=_Alu.mult, accum_out=stats[:, 0:1],
            ),
            after=i_m,
        )
        # depth 2: s2 = sum((m*p) * p)
        dv(
            nc.vector.scalar_tensor_tensor(
                out=sc1[:], in0=mp[:], scalar=1.0, in1=p[:],
                op0=_Alu.mult, op1=_Alu.mult, accum_out=stats[:, 1:2],
            ),
            after=i_mp,
        )
        # bridge: fold ACT's Square completion (s_act=2) into the s_dve count
        # so the matmul's single wait covers every stats producer; keeping all
        # s_dve updates on one engine keeps the count deterministic
        dv(nc.vector.wait_ge(s_act, 2))
        n_stats = ndve

        # --- epilogue: 4 sem-hops from totals to loss ---
        # hop 1 (all gated on s_pe only)
        i_va = dv(
            nc.vector.scalar_tensor_tensor(
                out=c(16, 19), in0=c(1, 4), scalar=acc[0:1, 1:2],
                in1=acc[0:1, 2:5], op0=_Alu.mult, op1=_Alu.add,
            )._wait_ge(s_pe, 1)
        )  # va = s2*(1,0,-1) + (n1,Tp,Tp2)
        i_ua = dv(
            nc.vector.scalar_tensor_tensor(
                out=c(19, 22), in0=c(13, 16), scalar=acc[0:1, 0:1],
                in1=c(4, 7), op0=_Alu.mult, op1=_Alu.add,
            )._wait_ge(s_pe, 1)
        )  # ua = s1*(0,-2,0) + (L,0,0)
        i_z = dv(
            nc.vector.scalar_tensor_tensor(
                out=c(22), in0=acc[0:1, 2:3], scalar=-1.0, in1=c(4),
                op0=_Alu.mult, op1=_Alu.add,
            )._wait_ge(s_pe, 1)
        )  # z = L - n1
        # hop 2
        i_V = dv(
            nc.vector.scalar_tensor_tensor(
                out=c(16, 19), in0=c(10, 13), scalar=acc[0:1, 0:1],
                in1=c(16, 19), op0=_Alu.mult, op1=_Alu.add,
            ),
            after=i_va,
        )  # V = s1*(-2,-1,0) + va = (alpha, gamma, delta)
        i_u = dv(
            nc.vector.scalar_tensor_tensor(
                out=c(19, 22), in0=c(7, 10), scalar=acc[0:1, 2:3],
                in1=c(19, 22), op0=_Alu.mult, op1=_Alu.add,
            ),
            after=i_ua,
        )  # u = n1*(-1,2,1) + ua = (z, 2*beta, n1)
        i_den = dv(
            nc.vector.scalar_tensor_tensor(
                out=c(23), in0=c(22), scalar=acc[0:1, 2:3], in1=c(0),
                op0=_Alu.mult, op1=_Alu.max,
            ),
            after=i_z,
        )  # den = max(z*n1, 1)
        # hop 3
        i_pair = dv(
            nc.vector.scalar_tensor_tensor(
                out=c(27, 30), in0=c(19, 22), scalar=1.0, in1=c(16, 19),
                op0=_Alu.mult, op1=_Alu.mult, accum_out=c(24),
            ),
            after=i_u,
        )  # pair = sum(u * V)
        i_rec = dv(nc.vector.reciprocal(c(25), c(23)), after=i_den)
        # hop 4
        dv(nc.vector.tensor_mul(c(26), c(24), c(25)), after=i_rec)
        n_all = ndve

        # sigmoid with fused row-sum: Tp lands as a stats column for free
        nc.scalar.activation(
            p[:], pred_v, _Act.Sigmoid, accum_out=stats[:, 3:4]
        )._wait_ge(s_in, 16).then_inc(s_act, 1)
        # Tp2 = sum(p^2) on ACT (Square shares the sigmoid table set). Its
        # completion is signaled on s_dve: the matmul's single wait below
        # covers both producers because DVE epilogue increments cannot occur
        # until s_pe fires, so s_dve == n_stats + 1 iff all DVE stats ops AND
        # this op are done.
        nc.scalar.wait_ge(s_act, 1)
        nc.scalar.activation(
            p2[:], p[:], _Act.Square, accum_out=stats[:, 4:5]
        ).then_inc(s_act, 1)

        nc.tensor.matmul(
            acc[0:1, 0:5], ones[:], stats[:, 0:5], start=True, stop=True
        )._wait_ge(s_dve, n_stats).then_inc(s_pe, 1)

        reg = nc.sync.alloc_register()
        nc.sync.reg_load(reg, c(26).bitcast(_i32))._wait_ge(s_dve, n_all)
        nc.sync.store(out_d[0:1, 0:1].bitcast(_i32), reg)
        assert tail == "none"

    entry = nc.main_func.blocks[0]
    raw = dma_inst.ins
    insts = entry.instructions
    insts.remove(raw)
    sp_drain = next(
        i
        for i, inst in enumerate(insts)
        if isinstance(inst, mybir.InstDrain) and inst.engine == mybir.EngineType.SP
    )
    insts.insert(sp_drain, raw)

    nc.compile()
    return nc


_build = _build_v4


def _pack(pred_Y, true_Y):
    xin = np.empty((P, 2 * F), dtype=np.float32)
    xin[:, 0:F] = np.ascontiguousarray(pred_Y, dtype=np.float32).reshape(P, F)
    xin[:, F : 2 * F] = (
        np.ascontiguousarray(true_Y, dtype=np.int32).reshape(P, F).view(np.float32)
    )
    return xin


def _run(pred_Y, true_Y, **hw_kwargs):
    global _built
    if _built is None:
        _built = _build()
    in_map = {"xin": _pack(pred_Y, true_Y)}
    res = run_bass_kernel_spmd(
        _built, [in_map] * N_CORES, list(range(N_CORES)), **hw_kwargs
    )
    out = np.asarray(res.results[0]["out"], dtype=np.float32).reshape(())
    return out, res


def kernel(pred_Y, true_Y):
    out, _ = _run(pred_Y, true_Y)
    return out
